# revision 42
# baseline (speedup 1.0000x reference)
"""Trainium2 Bass kernel for MinimalKAN forward (nn_MinimalKAN_Normalized).

Math:
  a = sigmoid(alpha)
  out = (1-a) * (x @ W.T + b) + (a/sqrt(I)) * (x @ C0 + x^2 @ C1 + x^3 @ C2)

Folding the alpha blend into the weights on the host gives exactly
  out = x @ A + x^2 @ B + x^3 @ C + b_eff
with A = (1-a) W.T + s C0, B = s C1, C = s C2, b_eff = (1-a) b, s = a/sqrt(I).

Device strategy (data-parallel over batch, 8 cores), per core 4096 rows.
The contraction index i sits on SBUF partitions; the host feeds x^T in fp16.
Mixed precision split by term magnitude:
  - linear term x @ A: fp16 matmuls (A host-scaled by S16=64 to clear the
    fp16 subnormal range), 4 accumulating matmuls per 128-row tile.
  - kan terms x^2 @ B + x^3 @ C: fp8(e4m3) matmuls in DoubleRow perf mode:
    lhsT [128, 2, 128] loads two k-planes (each PE cell holds 2 weights),
    K=256 per instruction at 1 col/cycle -> 2x the fp16 MAC rate.  B,C are
    tiny (~2e-4) so fp8 error lands well under tolerance; host-scaled by
    4096 to clear fp8 subnormals.  TRN e4m3 saturates at +-240: max|x|=5.4
    -> max|x^3| ~ 160, safe.
  - x^2 on ACT (Square, fp8 out), x^3 on GpSimd (x^2*x) at group
    granularity; PSUM merges + bias on DVE per tile; output stored bf16.
Per 128-row tile PE cost: 4*512 (fp16) + 4*512 (fp8 DR) = 4096 cycles vs
12*512 = 6144 all-fp16: ~57us PE floor at 2.3 GHz.
All HBM tensors are host-relayouted to [128 partitions, ...contiguous]:
descriptor GENERATION on the issuing sequencer costs ~5ns/descriptor
(~650ns per 128-descriptor kick), so kicks are as big as dependencies
allow: single kicks for the weights (ACT ring), per-group 512KB kicks
for x^T (SP ring) and outputs (ACT ring).  Exceptions tuned for the
pipeline head/tail: group 0 loads x^T and builds the basis per-tile so
compute starts after the first 128KB, and the last group drains its
outputs per-tile.  A burst of dummy 128-col matmuls bridges the initial
DMA fill so the PE p-state ramp completes on garbage data; an idle gap
there resets the ramp and costs ~2.5us of half-rate matmuls.
"""

import os
import numpy as np

import concourse.bass as bass
from concourse import bacc
import concourse.mybir as mybir
import concourse.tile as tile
from concourse.bass_utils import run_bass_kernel_spmd

N_CORES = 8
B, I, O = 32768, 512, 512
BS = B // N_CORES          # rows per core
P = 128
KS = I // P                # 4 contraction k-tiles per basis
N_TILES = BS // P          # 32 x 128-row tiles per core
G = int(os.environ.get("KAN_GROUP", "4"))     # tiles per x^2/x^3 group
N_GROUPS = N_TILES // G

S16 = 64.0                 # fp16 linear-weight host scale
S8A = 4096.0               # fp8 kan-weight host scale (x^2 and x^3 blocks)
N_WARM = int(os.environ.get("KAN_WARM", "40"))


def _build(repeat: int = 1) -> bass.Bass:
    f16 = mybir.dt.float16
    f8 = mybir.dt.float8e4
    f32 = mybir.dt.float32
    bf16 = mybir.dt.bfloat16
    sq = mybir.ActivationFunctionType.Square
    DR = mybir.MatmulPerfMode.DoubleRow
    mult = mybir.AluOpType.mult
    add = mybir.AluOpType.add

    nc = bacc.Bacc("TRN2", target_bir_lowering=False, debug=False,
                   num_devices=N_CORES)

    x_d = nc.dram_tensor("xt", [P, N_TILES, KS, P], f16,
                         kind="ExternalInput")
    wl_d = nc.dram_tensor("wlin", [P, KS, O], f16, kind="ExternalInput")
    wk_d = nc.dram_tensor("wkan", [P, 2 * KS, O], f8, kind="ExternalInput")
    b_d = nc.dram_tensor("bias", [P, O], f16, kind="ExternalInput")
    o_d = nc.dram_tensor("out", [P, N_TILES, O], bf16,
                         kind="ExternalOutput")

    with tile.TileContext(nc) as tc:
        with (
            tc.tile_pool(name="const", bufs=1) as const,
            tc.tile_pool(name="xin", bufs=3) as xin,
            tc.tile_pool(name="basis", bufs=3) as basis,
            tc.tile_pool(name="outp", bufs=3) as outp,
            tc.tile_pool(name="tmp", bufs=6) as tmpp,
            tc.tile_pool(name="ps_l", bufs=3, space="PSUM") as ps_l,
            tc.tile_pool(name="ps_k", bufs=3, space="PSUM") as ps_k,
            tc.tile_pool(name="ps_w", bufs=1, space="PSUM") as ps_w,
        ):
            # weights on the ACT ring; single kicks (descriptor generation
            # on the sequencer costs ~5ns/descriptor, so fewer+bigger
            # kicks beat k-sliced ones).
            wl_sb = const.tile([P, KS, O], f16)
            wl_mode = os.environ.get("KAN_WLSPLIT", "0")
            if wl_mode == "1":
                nc.scalar.dma_start(wl_sb[:, 0:2, :], wl_d[:, 0:2, :])
                nc.scalar.dma_start(wl_sb[:, 2:4, :], wl_d[:, 2:4, :])
            elif wl_mode == "2":
                # balance the first-group critical set across both rings:
                # half of wl rides the SP ring ahead of the x^T tiles
                nc.scalar.dma_start(wl_sb[:, 0:2, :], wl_d[:, 0:2, :])
                nc.sync.dma_start(wl_sb[:, 2:4, :], wl_d[:, 2:4, :])
            else:
                nc.scalar.dma_start(wl_sb[:], wl_d[:])
            wk_sb = const.tile([P, 2 * KS, O], f8)
            nc.scalar.dma_start(wk_sb[:], wk_d[:])
            bsb = const.tile([P, O], f16)

            # PE p-state warmup during the initial fill (short 128-col
            # matmuls; results discarded).
            warm = const.tile([P, P], f16)
            nc.vector.memset(warm[:], 0.0)
            po_w = ps_w.tile([P, P], f32, tag="po_w")
            for _ in range(N_WARM):
                nc.tensor.matmul(po_w[:], warm[:], warm[:],
                                 start=True, stop=True,
                                 skip_group_check=True)

            for g in [i for _ in range(repeat) for i in range(N_GROUPS)]:
                xT = xin.tile([P, G, KS, P], f16, tag="xT")
                if g == 0:
                    # first group: per-tile kicks so compute starts after
                    # the first 128KB
                    for j in range(G):
                        nc.sync.dma_start(xT[:, j], x_d[:, g * G + j])
                    nc.sync.dma_start(bsb[:], b_d[:])
                else:
                    nc.sync.dma_start(xT[:], x_d[:, g * G:(g + 1) * G])
                b8 = basis.tile([P, G, 2 * KS, P], f8, tag="b8")
                o_sb = outp.tile([P, G, O], bf16, tag="o_sb")
                if g == 0:
                    # first group: per-tile basis ops so the first kan
                    # matmuls don't wait on the whole group's x^T DMA
                    for j in range(G):
                        nc.scalar.activation(b8[:, j, 0:KS, :],
                                             xT[:, j], sq)
                        nc.gpsimd.tensor_mul(b8[:, j, KS:2 * KS, :],
                                             b8[:, j, 0:KS, :], xT[:, j])
                else:
                    nc.scalar.activation(b8[:, :, 0:KS, :], xT[:], sq)
                    nc.gpsimd.tensor_mul(b8[:, :, KS:2 * KS, :],
                                         b8[:, :, 0:KS, :], xT[:])
                if g == 0 and os.environ.get("KAN_LINFIRST", "0") == "1":
                    # group 0: run all linear matmuls first -- they only
                    # need wl + x^T, which land ~1.5us before wk/basis, so
                    # the PE stream starts earlier.  merge1 frees each
                    # po_l bank immediately; kan+merge2 follow.
                    tmps = []
                    for j in range(G):
                        po_l = ps_l.tile([P, O], f32, tag="po_l")
                        for k in range(KS):
                            nc.tensor.matmul(
                                po_l[:], xT[:, j, k, :], wl_sb[:, k, :],
                                start=(k == 0), stop=(k == KS - 1),
                                skip_group_check=True)
                        tmp = tmpp.tile([P, O], f32, tag="tmp")
                        nc.vector.scalar_tensor_tensor(
                            tmp[:], po_l[:], 1.0 / S16, bsb[:], mult, add)
                        tmps.append(tmp)
                    for j in range(G):
                        po_k = ps_k.tile([P, O], f32, tag="po_k")
                        for t in range(KS):
                            nc.tensor.matmul(
                                po_k[:],
                                b8[:, j, 2 * t:2 * t + 2, :],
                                wk_sb[:, 2 * t:2 * t + 2, :],
                                start=(t == 0), stop=(t == KS - 1),
                                perf_mode=DR, skip_group_check=True)
                        nc.vector.scalar_tensor_tensor(
                            o_sb[:, j, :], po_k[:], 1.0 / S8A, tmps[j],
                            mult, add)
                    nc.scalar.dma_start(o_d[:, 0:G, :], o_sb[:])
                    continue
                for j in range(G):
                    po_l = ps_l.tile([P, O], f32, tag="po_l")
                    for k in range(KS):
                        nc.tensor.matmul(
                            po_l[:], xT[:, j, k, :], wl_sb[:, k, :],
                            start=(k == 0), stop=(k == KS - 1),
                            skip_group_check=True)
                    po_k = ps_k.tile([P, O], f32, tag="po_k")
                    for t in range(KS):
                        nc.tensor.matmul(
                            po_k[:],
                            b8[:, j, 2 * t:2 * t + 2, :],
                            wk_sb[:, 2 * t:2 * t + 2, :],
                            start=(t == 0), stop=(t == KS - 1),
                            perf_mode=DR, skip_group_check=True)
                    tmp = tmpp.tile([P, O], f32, tag="tmp")
                    nc.vector.scalar_tensor_tensor(
                        tmp[:], po_l[:], 1.0 / S16, bsb[:], mult, add)
                    nc.vector.scalar_tensor_tensor(
                        o_sb[:, j, :], po_k[:], 1.0 / S8A, tmp[:], mult, add)
                    if g == N_GROUPS - 1:
                        # last group drains per-tile to shorten the tail
                        nc.scalar.dma_start(o_d[:, g * G + j, :],
                                            o_sb[:, j, :])
                if g != N_GROUPS - 1:
                    nc.scalar.dma_start(o_d[:, g * G:(g + 1) * G, :],
                                        o_sb[:])

    nc.compile()
    return nc


_NC_CACHE: dict[int, bass.Bass] = {}


def _get_nc(repeat: int = 1) -> bass.Bass:
    nc = _NC_CACHE.get(repeat)
    if nc is None:
        nc = _build(repeat)
        _NC_CACHE[repeat] = nc
    return nc


def _fold_weights(coeffs, W, b, alpha):
    a = 1.0 / (1.0 + np.exp(-np.float64(alpha)))
    s = a / np.sqrt(np.float64(I))
    A = (1.0 - a) * W.astype(np.float64).T + s * coeffs[:, :, 0].astype(np.float64)
    Bm = s * coeffs[:, :, 1].astype(np.float64)
    Cm = s * coeffs[:, :, 2].astype(np.float64)
    # [I, O] -> [P, KS, O] with row ks*P+p on partition p, slot ks
    wlin = (A * S16).astype(np.float16)
    wlin = np.ascontiguousarray(
        wlin.reshape(KS, P, O).transpose(1, 0, 2))
    f8np = mybir.dt.np(mybir.dt.float8e4)
    wkan = np.concatenate([Bm * S8A, Cm * S8A], axis=0)
    wkan = np.clip(wkan, -240.0, 240.0).astype(f8np)
    wkan = np.ascontiguousarray(
        wkan.reshape(2 * KS, P, O).transpose(1, 0, 2))
    b_eff = ((1.0 - a) * b.astype(np.float64)).astype(np.float16)
    bias_rep = np.ascontiguousarray(
        np.broadcast_to(b_eff[None, :], (P, O)))
    return wlin, wkan, bias_rep


def _make_in_maps(x, coeffs, W, b, alpha):
    wlin, wkan, bias_rep = _fold_weights(coeffs, W, b, alpha)
    x = np.asarray(x, dtype=np.float32)
    in_maps = []
    for c in range(N_CORES):
        shard = x[c * BS:(c + 1) * BS].astype(np.float16)
        # [BS, I] -> [P, N_TILES, KS, P]: xt[p, t, ks, c'] =
        # x[t*P+c', ks*P+p]
        xt = np.ascontiguousarray(
            shard.reshape(N_TILES, P, KS, P).transpose(3, 0, 2, 1))
        in_maps.append({
            "wlin": wlin, "wkan": wkan, "bias": bias_rep, "xt": xt,
        })
    return in_maps


def _unpack_out(raw):
    # [P, N_TILES, O] bf16 -> [BS, O] f32: row t*P + p
    return np.ascontiguousarray(
        np.asarray(raw).astype(np.float32).transpose(1, 0, 2)
    ).reshape(BS, O)


def _run(x, coeffs, W, b, alpha, trace=False):
    nc = _get_nc()
    in_maps = _make_in_maps(x, coeffs, W, b, alpha)
    res = run_bass_kernel_spmd(nc, in_maps, core_ids=list(range(N_CORES)),
                               trace=trace)
    out = np.concatenate([_unpack_out(r["out"]) for r in res.results], axis=0)
    return out, res


def kernel(x, coeffs, W, b, alpha):
    out, _ = _run(x, coeffs, W, b, alpha, trace=False)
    return out
